# revision 31
# baseline (speedup 1.0000x reference)
"""Multi-head attention (B=16, N=1024, C=384, H=6, D=64) on 8 trn2 cores.

Sharding: data-parallel over batch — each core computes 2 full batches.

The kernel is ScalarE-bound: softmax needs 96 exp calls of [128, 1024]
(~110us), every other engine's work must hide underneath them. Tile's
per-engine instruction order is static (emission order), so the code is
structured as a stream of 96 "exp slots", each emitting one S^T matmul pair
+ its exp, then draining ~800ns of deferred PE/DVE work from a filler queue
(previous chunk's PV, normalize, the next batch's qkv staging, the previous
batch's projection).

Per-core layout:
  - x is loaded naturally [n, c] (one DMA per 512-row half) and transposed
    on PE to xT [c, n] (bf16).
  - qkv^T = w_qkv.T @ x with q,k transposed (d on partitions, bf16) and
    v natural [n, d] bf16; biases folded into the PSUM->SBUF copies.
  - A head PAIR occupies partitions 0-63 / 64-127 of qkT, so S^T for both
    heads runs as two concurrent row-group matmuls (tile_position 0/64)
    writing [head0 | head1] halves of one [128, 1024] f32 PSUM tile; one
    exp call covers both (bf16 out).
  - PV with an augmented [V_h | 1] stationary tensor: row 64 of the output
    is the softmax denominator for free.
  - normalize: reciprocal of the sum row, K=1 matmul broadcasts it across
    64 partitions, row-scale into attnT (bf16).
  - proj: out[n, c2] = attnT.T @ w_proj + b_proj, stores merged 4 n-tiles
    per DMA.
"""

from collections import deque
from contextlib import ExitStack, nullcontext

import numpy as np

import concourse.bass as bass
import concourse.mybir as mybir
import concourse.tile as tile
from concourse import bacc
from concourse.bass_utils import run_bass_kernel_spmd
from concourse.masks import make_identity

f32 = mybir.dt.float32
f32r = mybir.dt.float32r
bf16 = mybir.dt.bfloat16
EXP = mybir.ActivationFunctionType.Exp

B, N, C = 16, 1024, 384
H, D = 6, 64
NCORES = 8
BL = B // NCORES           # batches per core
HP = H // 2                # head pairs
SCALE = D ** -0.5
P = 128
NT = N // P                # 8 n-tiles
CT = C // P                # 3 c-tiles
KT = N // P                # 8 k-tiles in attention
QC = 2                     # 512-wide q chunks
QW = N // QC               # 512


def _r(ap, dt=f32r):
    return ap.bitcast(dt)


def build_nc(repeat=1, hwloop=False, skip=()):
    """skip: diagnostic variants for differential timing."""
    skip = frozenset(skip)
    nc = bacc.Bacc("TRN2", target_bir_lowering=False, debug=False)

    x_d = nc.dram_tensor("x", [BL, N, C], f32, kind="ExternalInput").ap()
    wqkv_d = nc.dram_tensor("w_qkv", [C, 3 * C], f32, kind="ExternalInput").ap()
    bqkv_d = nc.dram_tensor("b_qkv", [3 * C], f32, kind="ExternalInput").ap()
    wproj_d = nc.dram_tensor("w_proj", [C, C], f32, kind="ExternalInput").ap()
    bproj_d = nc.dram_tensor("b_proj", [C], f32, kind="ExternalInput").ap()
    out_d = nc.dram_tensor("out", [BL, N, C], f32, kind="ExternalOutput").ap()

    with tile.TileContext(nc) as tc, ExitStack() as ctx:
        consts = ctx.enter_context(tc.tile_pool(name="consts", bufs=1))
        big = ctx.enter_context(tc.tile_pool(name="big", bufs=1))
        work4 = ctx.enter_context(tc.tile_pool(name="work4", bufs=4))
        db = ctx.enter_context(tc.tile_pool(name="db", bufs=2))
        ps_st = ctx.enter_context(tc.tile_pool(name="ps_st", bufs=2, space="PSUM"))
        ps_pv = ctx.enter_context(tc.tile_pool(name="ps_pv", bufs=2, space="PSUM"))
        ps_wk = ctx.enter_context(tc.tile_pool(name="ps_wk", bufs=2, space="PSUM"))

        # ---- constants ----
        ident = consts.tile([P, P], f32)
        make_identity(nc, ident)
        ones64 = consts.tile([P, 64], f32)
        nc.vector.memset(ones64[:], 1.0)

        def emit_x_load(b):
            xb = big.tile([P, NT, C], f32, tag=f"xb{b}")
            xr = x_d[b].rearrange("(t p) c -> p t c", p=P)
            for half in (0, 1):
                nc.sync.dma_start(
                    xb[:, half * 4:(half + 1) * 4, :],
                    xr[:, half * 4:(half + 1) * 4, :])
            return xb

        # x(0) first so its transposes can start ASAP; weight loads follow
        # on the DMA rings behind it.
        xb0 = emit_x_load(0)
        bqk_sb = consts.tile([P, 6], f32)
        nc.sync.dma_start(bqk_sb[:], bqkv_d[0:768].rearrange("(t p) -> p t", p=P))
        wqr = wqkv_d.rearrange("(kt p) m -> p kt m", p=P)
        wqk_raw = big.tile([P, CT, 768], f32, tag="wraw")
        nc.sync.dma_start(wqk_raw[:], wqr[:, :, 0:768])
        wqk_sb = consts.tile([P, CT, 768], bf16)
        nc.vector.tensor_copy(wqk_sb[:], wqk_raw[:])
        bv_sb = consts.tile([P, C], f32)
        nc.sync.dma_start(bv_sb[:], bqkv_d[None, 768:1152].to_broadcast((P, C)))
        wv_raw = big.tile([P, CT, 768], f32, tag="wraw")
        nc.sync.dma_start(wv_raw[:, :, 0:C], wqr[:, :, 768:1152])
        wv_sb = consts.tile([P, CT, C], bf16)
        nc.vector.tensor_copy(wv_sb[:], wv_raw[:, :, 0:C])
        wp_raw = big.tile([P, CT, 768], f32, tag="wraw")
        nc.sync.dma_start(
            wp_raw[:, :, 0:C], wproj_d.rearrange("(kt p) m -> p kt m", p=P))
        wproj_sb = consts.tile([P, CT, C], bf16)
        nc.vector.tensor_copy(wproj_sb[:], wp_raw[:, :, 0:C])
        bp_sb = consts.tile([P, C], f32)
        nc.sync.dma_start(bp_sb[:], bproj_d[None, :].to_broadcast((P, C)))

        # ---- deferred-work queues (cost_ns, closure) ----
        # fast: latency-critical (PV, normalize, v of the batch being
        # consumed); bulk: next-batch staging and projection.
        fast = deque()
        bulk = deque()

        def drain(fast_ns, bulk_ns):
            while fast and fast_ns > 0:
                cost, f = fast.popleft()
                f()
                fast_ns -= cost
            while bulk and bulk_ns > 0:
                cost, f = bulk.popleft()
                f()
                bulk_ns -= cost

        def drain_all():
            while fast:
                fast.popleft()[1]()
            while bulk:
                bulk.popleft()[1]()

        def make_ab_units(b, xb):
            """Transpose + qkv units for batch b; returns (qkT, v_sb, units).

            Unit order satisfies intra-batch deps: half-0 transposes, then
            the q/k slices of head pair 0 for q-chunk 0, etc.
            """
            if "ab" in skip:
                qkT = db.tile([P, 6, N], bf16, tag="qkT")
                nc.vector.memset(qkT[:], 0.01)
                v_sb = db.tile([P, NT, H * (D + 1)], bf16, tag="v_sb")
                nc.vector.memset(v_sb[:], 1.0)
                return qkT, v_sb, [], []
            xT = db.tile([P, CT, N], bf16, tag="xT")
            qkT = db.tile([P, 6, N], bf16, tag="qkT")
            v_sb = db.tile([P, NT, H * (D + 1)], bf16, tag="v_sb")

            def t_unit(half, ct):
                def f():
                    g = ps_wk.tile([P, QW], f32, tag="wk")
                    for j in range(4):
                        nc.tensor.transpose(
                            g[:, j * P:(j + 1) * P],
                            xb[:, half * 4 + j, ct * P:(ct + 1) * P],
                            ident[:],
                        )
                    nc.vector.tensor_copy(
                        xT[:, ct, half * QW:(half + 1) * QW], g[:])
                return (350, f)

            def qk_unit(m, ch):
                def f():
                    ps = ps_wk.tile([P, QW], f32, tag="wk")
                    for kt in range(CT):
                        nc.tensor.matmul(
                            ps[:],
                            lhsT=wqk_sb[:, kt, m * P:(m + 1) * P],
                            rhs=xT[:, kt, ch * QW:(ch + 1) * QW],
                            start=(kt == 0), stop=(kt == CT - 1),
                        )
                    nc.vector.tensor_scalar_add(
                        qkT[:, m, ch * QW:(ch + 1) * QW], ps[:],
                        bqk_sb[:, m:m + 1])
                return (660, f)

            def ones_unit():
                def f():
                    nc.vector.memset(
                        v_sb[:].rearrange(
                            "p t (h e) -> p t h e", e=D + 1)[:, :, :, D:],
                        1.0)
                return (30, f)

            def v_unit(nt):
                def f():
                    ps = ps_wk.tile([P, QW], f32, tag="wk")
                    for kt in range(CT):
                        nc.tensor.matmul(
                            ps[:, 0:C],
                            lhsT=xT[:, kt, nt * P:(nt + 1) * P],
                            rhs=wv_sb[:, kt, :],
                            start=(kt == 0), stop=(kt == CT - 1),
                        )
                    nc.vector.tensor_tensor(
                        v_sb[:, nt].rearrange(
                            "p (h e) -> p h e", e=D + 1)[:, :, 0:D],
                        ps[:, 0:C].rearrange("p (h e) -> p h e", e=D),
                        bv_sb[:].rearrange("p (h e) -> p h e", e=D),
                        mybir.AluOpType.add,
                    )
                return (500, f)

            units = [t_unit(0, 0), t_unit(0, 1), t_unit(0, 2),
                     qk_unit(0, 0), qk_unit(3, 0),
                     t_unit(1, 0), t_unit(1, 1), t_unit(1, 2),
                     qk_unit(3, 1), qk_unit(0, 1),
                     qk_unit(1, 0), qk_unit(4, 0), qk_unit(1, 1), qk_unit(4, 1),
                     qk_unit(2, 0), qk_unit(5, 0), qk_unit(2, 1), qk_unit(5, 1)]
            v_units = [ones_unit()] + [v_unit(nt) for nt in range(NT)]
            return qkT, v_sb, units, v_units

        def push_pv(hp, ch, pt, v_sb, aus):
            if "pv" in skip:
                return
            po = [None, None]

            def mk_step(kt):
                def f():
                    if kt == 0:
                        po[0] = ps_pv.tile([65, QW], f32, tag="pv", name="po0")
                        po[1] = ps_pv.tile([65, QW], f32, tag="pv", name="po1")
                    for head_i in (0, 1):
                        head = 2 * hp + head_i
                        nc.tensor.matmul(
                            po[head_i][:],
                            lhsT=v_sb[:, kt,
                                      head * (D + 1):(head + 1) * (D + 1)],
                            rhs=pt[:, kt, head_i, :],
                            start=(kt == 0), stop=(kt == KT - 1),
                        )
                return f

            def fin():
                for head_i in (0, 1):
                    nc.vector.tensor_copy(
                        aus[head_i][:, ch * QW:(ch + 1) * QW],
                        po[head_i][0:65, :])

            fast.extend((440, mk_step(kt)) for kt in range(KT))
            fast.append((30, fin))

        def push_norm(hp, aus, attnT):
            if "pv" in skip:
                return

            def recip(head_i):
                def f():
                    with nc.allow_low_precision(
                            reason="f32r rounding of softmax recip"):
                        nc.vector.reciprocal(
                            aus[head_i][64:65, :], aus[head_i][64:65, :])
                return f

            an = db.tile([64, N], bf16, tag="attnN")

            def rbmul(head_i, ch, dst):
                def f():
                    au = aus[head_i]
                    rb = ps_pv.tile([P, QW], f32, tag="pv")
                    nc.tensor.matmul(
                        rb[0:64, :],
                        lhsT=_r(ones64[64:65, :]),
                        rhs=_r(au[64:65, ch * QW:(ch + 1) * QW]),
                        tile_position=(64, 0),
                        start=True, stop=True,
                    )
                    nc.vector.tensor_mul(
                        dst[:, ch * QW:(ch + 1) * QW],
                        au[0:64, ch * QW:(ch + 1) * QW],
                        rb[0:64, :],
                    )
                return f

            def an_dma():
                nc.sync.dma_start(attnT[64:128, hp, :], an[:])

            fast.append((20, recip(0)))
            fast.append((20, recip(1)))
            fast.append((250, rbmul(0, 0, attnT[0:64, hp, :])))
            fast.append((250, rbmul(0, 1, attnT[0:64, hp, :])))
            fast.append((250, rbmul(1, 0, an[:])))
            fast.append((250, rbmul(1, 1, an[:])))
            fast.append((20, an_dma))

        def stage_c(b, qkT, v_sb, attnT, needs_bulk=False):
            """48 exp slots for batch b; PV/norm one chunk-group behind.

            needs_bulk: this batch's staging units live in the bulk queue —
            force-emit any leftovers first, since the slots' S^T instructions
            reference their outputs and Tile orders edges by emission.
            """
            if needs_bulk:
                while bulk:
                    bulk.popleft()[1]()
            if "pv" in skip:
                nc.vector.memset(attnT[:], 0.01)

            def emit_st(hp, ch, kt):
                st = ps_st.tile([P, N], f32, tag="st")
                nc.tensor.matmul(
                    st[:, 0:QW],
                    lhsT=qkT[0:64, 3 + hp, kt * P:(kt + 1) * P],
                    rhs=qkT[0:64, hp, ch * QW:(ch + 1) * QW],
                    tile_position=(0, 0), start=True, stop=True,
                )
                nc.tensor.matmul(
                    st[:, QW:N],
                    lhsT=qkT[64:128, 3 + hp, kt * P:(kt + 1) * P],
                    rhs=qkT[64:128, hp, ch * QW:(ch + 1) * QW],
                    tile_position=(64, 0), start=True, stop=True,
                )
                return st

            slots = [(hp, ch, kt)
                     for hp in range(HP) for ch in range(QC)
                     for kt in range(KT)]
            aus = None
            pt = None
            for i, (hp, ch, kt) in enumerate(slots):
                if kt == 0:
                    pt = big.tile([P, KT, 2, QW], bf16, tag="pt",
                                  bufs=3, name="pt")
                    if "exp" in skip:
                        nc.vector.memset(pt[:], 0.001)
                if kt == 0 and ch == 0:
                    au0 = work4.tile([65, N], f32r, tag="attnU")
                    au1 = work4.tile([65, N], f32r, tag="attnU")
                    aus = (au0, au1)
                if "st" not in skip:
                    st = emit_st(hp, ch, kt)
                    if "exp" not in skip:
                        nc.scalar.activation(
                            pt[:, kt, :, :], st[:],
                            EXP, scale=SCALE)
                drain(700, 350)
                if kt == KT - 1:
                    push_pv(hp, ch, pt, v_sb, aus)
                    if ch == QC - 1:
                        push_norm(hp, aus, attnT)

        def push_d(b, attnT):
            ob = [None]

            def d_unit(nt):
                def f():
                    if nt % 4 == 0:
                        ob[0] = db.tile([P, 4, C], f32, tag="ob", name="ob")
                    if "proj" in skip:
                        nc.vector.tensor_add(
                            ob[0][:, nt % 4, :], bp_sb[:], bp_sb[:])
                    else:
                        ps = ps_wk.tile([P, QW], f32, tag="wk")
                        for ct in range(CT):
                            nc.tensor.matmul(
                                ps[:, 0:C],
                                lhsT=attnT[:, ct, nt * P:(nt + 1) * P],
                                rhs=wproj_sb[:, ct, :],
                                start=(ct == 0), stop=(ct == CT - 1),
                            )
                        nc.vector.tensor_add(
                            ob[0][:, nt % 4, :], ps[:, 0:C], bp_sb[:])
                    if nt % 4 == 3 and "out" not in skip:
                        nc.sync.dma_start(
                            out_d[b].rearrange("(t p) c -> p t c", p=P)[
                                :, nt - 3:nt + 1, :],
                            ob[0][:],
                        )
                return (520, f)

            fast.extend(d_unit(nt) for nt in range(NT))

        loop_ctx = tc.For_i(0, repeat, 1) if hwloop else nullcontext(None)
        with loop_ctx:
            for rep in range(1 if hwloop else repeat):
                fast.clear()
                bulk.clear()
                xb0_r = emit_x_load(0) if (hwloop or rep > 0) else xb0
                qkT0, v0, units0, vu0 = make_ab_units(0, xb0_r)
                # run just enough inline for head-pair 0's S^T to start
                for cost, f in units0[:10]:
                    f()
                fast.extend(units0[10:])
                fast.extend(vu0)
                attnT0 = big.tile([P, HP, N], bf16, tag="attnT0")
                xb1 = emit_x_load(1)
                qkT1, v1, units1, vu1 = make_ab_units(1, xb1)
                bulk.extend(units1)
                stage_c(0, qkT0, v0, attnT0)
                attnT1 = big.tile([P, HP, N], bf16, tag="attnT1")
                fast.extend(vu1)
                push_d(0, attnT0)
                stage_c(1, qkT1, v1, attnT1, needs_bulk=True)
                push_d(1, attnT1)
                drain_all()

    nc.compile()
    return nc


_NC_CACHE = {}


def _get_nc():
    if "nc" not in _NC_CACHE:
        _NC_CACHE["nc"] = build_nc()
    return _NC_CACHE["nc"]


def kernel(x, w_qkv, b_qkv, w_proj, b_proj):
    x = np.asarray(x, dtype=np.float32)
    w_qkv = np.asarray(w_qkv, dtype=np.float32)
    b_qkv = np.asarray(b_qkv, dtype=np.float32)
    w_proj = np.asarray(w_proj, dtype=np.float32)
    b_proj = np.asarray(b_proj, dtype=np.float32)

    nc = _get_nc()
    in_maps = [
        {
            "x": np.ascontiguousarray(x[i * BL:(i + 1) * BL]),
            "w_qkv": w_qkv,
            "b_qkv": b_qkv,
            "w_proj": w_proj,
            "b_proj": b_proj,
        }
        for i in range(NCORES)
    ]
    res = run_bass_kernel_spmd(nc, in_maps, list(range(NCORES)))
    return np.concatenate([res.results[i]["out"] for i in range(NCORES)], axis=0)


# revision 35
# speedup vs baseline: 1.1634x; 1.1634x over previous
"""Multi-head attention (B=16, N=1024, C=384, H=6, D=64) on 8 trn2 cores.

Sharding: data-parallel over batch — each core computes 2 full batches.

The kernel is ScalarE-bound: softmax needs 96 exp calls of [128, 1024]
(~110us), every other engine's work must hide underneath them. Tile's
per-engine instruction order is static (emission order), so the code is
structured as a stream of 96 "exp slots", each emitting one S^T matmul pair
+ its exp, then draining ~800ns of deferred PE/DVE work from a filler queue
(previous chunk's PV, normalize, the next batch's qkv staging, the previous
batch's projection).

Per-core layout:
  - x is loaded naturally [n, c] (one DMA per 512-row half) and transposed
    on PE to xT [c, n] (bf16).
  - qkv^T = w_qkv.T @ x with q,k transposed (d on partitions, bf16) and
    v natural [n, d] bf16; biases folded into the PSUM->SBUF copies.
  - A head PAIR occupies partitions 0-63 / 64-127 of qkT, so S^T for both
    heads runs as two concurrent row-group matmuls (tile_position 0/64)
    writing [head0 | head1] halves of one [128, 1024] f32 PSUM tile; one
    exp call covers both (bf16 out).
  - PV with an augmented [V_h | 1] stationary tensor: row 64 of the output
    is the softmax denominator for free.
  - normalize: reciprocal of the sum row, K=1 matmul broadcasts it across
    64 partitions, row-scale into attnT (bf16).
  - proj: out[n, c2] = attnT.T @ w_proj + b_proj, stores merged 4 n-tiles
    per DMA.
"""

from collections import deque
from contextlib import ExitStack, nullcontext

import numpy as np

import concourse.bass as bass
import concourse.mybir as mybir
import concourse.tile as tile
from concourse import bacc
from concourse.bass_utils import run_bass_kernel_spmd
from concourse.masks import make_identity

f32 = mybir.dt.float32
f32r = mybir.dt.float32r
bf16 = mybir.dt.bfloat16
EXP = mybir.ActivationFunctionType.Exp

B, N, C = 16, 1024, 384
H, D = 6, 64
NCORES = 8
BL = B // NCORES           # batches per core
HP = H // 2                # head pairs
SCALE = D ** -0.5
P = 128
NT = N // P                # 8 n-tiles
CT = C // P                # 3 c-tiles
KT = N // P                # 8 k-tiles in attention
QC = 2                     # 512-wide q chunks
QW = N // QC               # 512


def _r(ap, dt=f32r):
    return ap.bitcast(dt)


def build_nc(repeat=1, hwloop=False, skip=()):
    """skip: diagnostic variants for differential timing."""
    skip = frozenset(skip)
    nc = bacc.Bacc("TRN2", target_bir_lowering=False, debug=False)

    x_d = nc.dram_tensor("x", [BL, N, C], f32, kind="ExternalInput").ap()
    wqkv_d = nc.dram_tensor("w_qkv", [C, 3 * C], f32, kind="ExternalInput").ap()
    bqkv_d = nc.dram_tensor("b_qkv", [3 * C], f32, kind="ExternalInput").ap()
    wproj_d = nc.dram_tensor("w_proj", [C, C], f32, kind="ExternalInput").ap()
    bproj_d = nc.dram_tensor("b_proj", [C], f32, kind="ExternalInput").ap()
    out_d = nc.dram_tensor("out", [BL, N, C], f32, kind="ExternalOutput").ap()

    with tile.TileContext(nc) as tc, ExitStack() as ctx:
        consts = ctx.enter_context(tc.tile_pool(name="consts", bufs=1))
        big = ctx.enter_context(tc.tile_pool(name="big", bufs=1))
        work4 = ctx.enter_context(tc.tile_pool(name="work4", bufs=4))
        db = ctx.enter_context(tc.tile_pool(name="db", bufs=2))
        ps_st = ctx.enter_context(tc.tile_pool(name="ps_st", bufs=2, space="PSUM"))
        ps_pv = ctx.enter_context(tc.tile_pool(name="ps_pv", bufs=2, space="PSUM"))
        ps_wk = ctx.enter_context(tc.tile_pool(name="ps_wk", bufs=2, space="PSUM"))

        # ---- constants ----
        ident = consts.tile([P, P], f32)
        make_identity(nc, ident)
        ones64 = consts.tile([P, 64], f32)
        nc.vector.memset(ones64[:], 1.0)

        def emit_x_load(b):
            xb = big.tile([P, NT, C], f32, tag=f"xb{b}")
            xr = x_d[b].rearrange("(t p) c -> p t c", p=P)
            for half in (0, 1):
                nc.sync.dma_start(
                    xb[:, half * 4:(half + 1) * 4, :],
                    xr[:, half * 4:(half + 1) * 4, :])
            return xb

        # x(0) first so its transposes can start ASAP; weight loads follow
        # on the DMA rings behind it.
        xb0 = emit_x_load(0)
        bqk_sb = consts.tile([P, 6], f32)
        nc.sync.dma_start(bqk_sb[:], bqkv_d[0:768].rearrange("(t p) -> p t", p=P))
        wqr = wqkv_d.rearrange("(kt p) m -> p kt m", p=P)
        wqk_raw = big.tile([P, CT, 768], f32, tag="wraw")
        nc.sync.dma_start(wqk_raw[:], wqr[:, :, 0:768])
        wqk_sb = consts.tile([P, CT, 768], bf16)
        nc.vector.tensor_copy(wqk_sb[:], wqk_raw[:])
        bv_sb = consts.tile([P, C], f32)
        nc.sync.dma_start(bv_sb[:], bqkv_d[None, 768:1152].to_broadcast((P, C)))
        wv_raw = big.tile([P, CT, 768], f32, tag="wraw")
        nc.sync.dma_start(wv_raw[:, :, 0:C], wqr[:, :, 768:1152])
        wv_sb = consts.tile([P, CT, C], bf16)
        nc.vector.tensor_copy(wv_sb[:], wv_raw[:, :, 0:C])
        wp_raw = big.tile([P, CT, 768], f32, tag="wraw")
        nc.sync.dma_start(
            wp_raw[:, :, 0:C], wproj_d.rearrange("(kt p) m -> p kt m", p=P))
        wproj_sb = consts.tile([P, CT, C], bf16)
        nc.vector.tensor_copy(wproj_sb[:], wp_raw[:, :, 0:C])
        bp_sb = consts.tile([P, C], f32)
        nc.sync.dma_start(bp_sb[:], bproj_d[None, :].to_broadcast((P, C)))

        # ---- deferred-work queues (cost_ns, closure) ----
        # fast: latency-critical (PV, normalize, v of the batch being
        # consumed); bulk: next-batch staging and projection.
        fast = deque()
        bulk = deque()

        def drain(fast_ns, bulk_ns):
            while fast and fast_ns > 0:
                cost, f = fast.popleft()
                f()
                fast_ns -= cost
            while bulk and bulk_ns > 0:
                cost, f = bulk.popleft()
                f()
                bulk_ns -= cost

        def drain_all():
            while fast:
                fast.popleft()[1]()
            while bulk:
                bulk.popleft()[1]()

        def make_ab_units(b, xb):
            """Transpose + qkv units for batch b; returns (qkT, v_sb, units).

            Unit order satisfies intra-batch deps: half-0 transposes, then
            the q/k slices of head pair 0 for q-chunk 0, etc.
            """
            if "ab" in skip:
                qkT = db.tile([P, 6, N], bf16, tag="qkT")
                nc.vector.memset(qkT[:], 0.01)
                v_sb = db.tile([P, NT, H * (D + 1)], bf16, tag="v_sb")
                nc.vector.memset(v_sb[:], 1.0)
                return qkT, v_sb, [], []
            xT = db.tile([P, CT, N], bf16, tag="xT")
            qkT = db.tile([P, 6, N], bf16, tag="qkT")
            v_sb = db.tile([P, NT, H * (D + 1)], bf16, tag="v_sb")

            def t_unit(half, ct):
                def f():
                    g = ps_wk.tile([P, QW], f32, tag="wk")
                    for j in range(4):
                        nc.tensor.transpose(
                            g[:, j * P:(j + 1) * P],
                            xb[:, half * 4 + j, ct * P:(ct + 1) * P],
                            ident[:],
                        )
                    nc.vector.tensor_copy(
                        xT[:, ct, half * QW:(half + 1) * QW], g[:])
                return (350, f)

            def qk_unit(m, ch):
                def f():
                    ps = ps_wk.tile([P, QW], f32, tag="wk")
                    for kt in range(CT):
                        nc.tensor.matmul(
                            ps[:],
                            lhsT=wqk_sb[:, kt, m * P:(m + 1) * P],
                            rhs=xT[:, kt, ch * QW:(ch + 1) * QW],
                            start=(kt == 0), stop=(kt == CT - 1),
                        )
                    nc.vector.tensor_scalar_add(
                        qkT[:, m, ch * QW:(ch + 1) * QW], ps[:],
                        bqk_sb[:, m:m + 1])
                return (660, f)

            def ones_unit():
                def f():
                    nc.vector.memset(
                        v_sb[:].rearrange(
                            "p t (h e) -> p t h e", e=D + 1)[:, :, :, D:],
                        1.0)
                return (30, f)

            def v_unit(nt):
                def f():
                    ps = ps_wk.tile([P, QW], f32, tag="wk")
                    for kt in range(CT):
                        nc.tensor.matmul(
                            ps[:, 0:C],
                            lhsT=xT[:, kt, nt * P:(nt + 1) * P],
                            rhs=wv_sb[:, kt, :],
                            start=(kt == 0), stop=(kt == CT - 1),
                        )
                    nc.vector.tensor_tensor(
                        v_sb[:, nt].rearrange(
                            "p (h e) -> p h e", e=D + 1)[:, :, 0:D],
                        ps[:, 0:C].rearrange("p (h e) -> p h e", e=D),
                        bv_sb[:].rearrange("p (h e) -> p h e", e=D),
                        mybir.AluOpType.add,
                    )
                return (500, f)

            units = [t_unit(0, 0), t_unit(0, 1), t_unit(0, 2),
                     qk_unit(0, 0), qk_unit(3, 0),
                     t_unit(1, 0), t_unit(1, 1), t_unit(1, 2),
                     qk_unit(3, 1), qk_unit(0, 1),
                     qk_unit(1, 0), qk_unit(4, 0), qk_unit(1, 1), qk_unit(4, 1),
                     qk_unit(2, 0), qk_unit(5, 0), qk_unit(2, 1), qk_unit(5, 1)]
            v_units = [ones_unit()] + [v_unit(nt) for nt in range(NT)]
            return qkT, v_sb, units, v_units

        def push_pv(hp, ch, pt, v_sb, aus, rbbs):
            if "pv" in skip:
                return
            po = [None, None]

            def mk_step(kt):
                def f():
                    if kt == 0:
                        po[0] = ps_pv.tile([65, QW], f32, tag="pv", name="po0")
                        po[1] = ps_pv.tile([65, QW], f32, tag="pv", name="po1")
                    for head_i in (0, 1):
                        head = 2 * hp + head_i
                        nc.tensor.matmul(
                            po[head_i][:],
                            lhsT=v_sb[:, kt,
                                      head * (D + 1):(head + 1) * (D + 1)],
                            rhs=pt[:, kt, head_i, :],
                            start=(kt == 0), stop=(kt == KT - 1),
                        )
                return f

            def fin():
                sl = slice(ch * QW, (ch + 1) * QW)
                for head_i in (0, 1):
                    nc.vector.tensor_copy(aus[head_i][:, sl], po[head_i][0:65, :])
                # reciprocal of this chunk's denominator row, then an async
                # DMA replicates it across 64 partitions for the row-scale
                for head_i in (0, 1):
                    with nc.allow_low_precision(
                            reason="f32r rounding of softmax recip"):
                        nc.vector.reciprocal(
                            aus[head_i][64:65, sl], aus[head_i][64:65, sl])

            fast.extend((440, mk_step(kt)) for kt in range(KT))
            fast.append((80, fin))

        def push_norm(hp, aus, attnT, rbbs):
            if "pv" in skip or "norm" in skip:
                return

            an = db.tile([64, N], bf16, tag="attnN")

            def rbmul(head_i, ch, dst):
                def f():
                    au = aus[head_i]
                    rb = ps_pv.tile([P, QW], f32, tag="pv", name="rb")
                    nc.tensor.matmul(
                        rb[0:64, :],
                        lhsT=_r(ones64[64:65, :]),
                        rhs=_r(au[64:65, ch * QW:(ch + 1) * QW]),
                        tile_position=(64, 0),
                        start=True, stop=True,
                    )
                    nc.vector.tensor_mul(
                        dst[:, ch * QW:(ch + 1) * QW],
                        au[0:64, ch * QW:(ch + 1) * QW],
                        rb[0:64, :],
                    )
                return f

            def an_dma():
                nc.sync.dma_start(attnT[64:128, hp, :], an[:])

            fast.append((250, rbmul(0, 0, attnT[0:64, hp, :])))
            fast.append((250, rbmul(0, 1, attnT[0:64, hp, :])))
            fast.append((250, rbmul(1, 0, an[:])))
            fast.append((250, rbmul(1, 1, an[:])))
            fast.append((20, an_dma))

        def stage_c(b, qkT, v_sb, attnT, needs_bulk=False):
            """48 exp slots for batch b; PV/norm one chunk-group behind.

            needs_bulk: this batch's staging units live in the bulk queue —
            force-emit any leftovers first, since the slots' S^T instructions
            reference their outputs and Tile orders edges by emission.
            """
            if needs_bulk:
                while bulk:
                    bulk.popleft()[1]()
            if "pv" in skip or "norm" in skip:
                nc.vector.memset(attnT[:], 0.01)

            def emit_st(hp, ch, kt):
                st = ps_st.tile([P, N], f32, tag="st")
                nc.tensor.matmul(
                    st[:, 0:QW],
                    lhsT=qkT[0:64, 3 + hp, kt * P:(kt + 1) * P],
                    rhs=qkT[0:64, hp, ch * QW:(ch + 1) * QW],
                    tile_position=(0, 0), start=True, stop=True,
                )
                nc.tensor.matmul(
                    st[:, QW:N],
                    lhsT=qkT[64:128, 3 + hp, kt * P:(kt + 1) * P],
                    rhs=qkT[64:128, hp, ch * QW:(ch + 1) * QW],
                    tile_position=(64, 0), start=True, stop=True,
                )
                return st

            slots = [(hp, ch, kt)
                     for hp in range(HP) for ch in range(QC)
                     for kt in range(KT)]
            aus = None
            pt = None
            for i, (hp, ch, kt) in enumerate(slots):
                if kt == 0:
                    pt = big.tile([P, KT, 2, QW], bf16, tag="pt",
                                  bufs=4, name="pt")
                    stage_c.pt_allocs = getattr(stage_c, "pt_allocs", 0) + 1
                    if "exp" in skip and stage_c.pt_allocs <= 4:
                        nc.vector.memset(pt[:], 0.001)
                if kt == 0 and ch == 0:
                    au0 = work4.tile([65, N], f32r, tag="attnU")
                    au1 = work4.tile([65, N], f32r, tag="attnU")
                    aus = (au0, au1)
                    rbbs = {}
                if "st" not in skip:
                    st = emit_st(hp, ch, kt)
                    if "exp" not in skip:
                        nc.scalar.activation(
                            pt[:, kt, :, :], st[:],
                            EXP, scale=SCALE)
                drain(700, 350)
                if kt == KT - 1:
                    push_pv(hp, ch, pt, v_sb, aus, rbbs)
                    if ch == QC - 1:
                        push_norm(hp, aus, attnT, rbbs)

        def push_d(b, attnT):
            ob = [None]

            def d_unit(nt):
                def f():
                    if nt % 4 == 0:
                        ob[0] = db.tile([P, 4, C], f32, tag="ob", name="ob",
                                        bufs=1)
                    if "proj" in skip:
                        nc.vector.tensor_add(
                            ob[0][:, nt % 4, :], bp_sb[:], bp_sb[:])
                    else:
                        ps = ps_wk.tile([P, QW], f32, tag="wk")
                        for ct in range(CT):
                            nc.tensor.matmul(
                                ps[:, 0:C],
                                lhsT=attnT[:, ct, nt * P:(nt + 1) * P],
                                rhs=wproj_sb[:, ct, :],
                                start=(ct == 0), stop=(ct == CT - 1),
                            )
                        nc.vector.tensor_add(
                            ob[0][:, nt % 4, :], ps[:, 0:C], bp_sb[:])
                    if nt % 4 == 3 and "out" not in skip:
                        nc.sync.dma_start(
                            out_d[b].rearrange("(t p) c -> p t c", p=P)[
                                :, nt - 3:nt + 1, :],
                            ob[0][:],
                        )
                return (520, f)

            fast.extend(d_unit(nt) for nt in range(NT))

        loop_ctx = tc.For_i(0, repeat, 1) if hwloop else nullcontext(None)
        with loop_ctx:
            for rep in range(1 if hwloop else repeat):
                fast.clear()
                bulk.clear()
                xb0_r = emit_x_load(0) if (hwloop or rep > 0) else xb0
                qkT0, v0, units0, vu0 = make_ab_units(0, xb0_r)
                # run just enough inline for head-pair 0's S^T to start
                for cost, f in units0[:10]:
                    f()
                fast.extend(units0[10:])
                fast.extend(vu0)
                attnT0 = big.tile([P, HP, N], bf16, tag="attnT0")
                xb1 = emit_x_load(1)
                qkT1, v1, units1, vu1 = make_ab_units(1, xb1)
                bulk.extend(units1)
                stage_c(0, qkT0, v0, attnT0)
                attnT1 = big.tile([P, HP, N], bf16, tag="attnT1")
                fast.extend(vu1)
                push_d(0, attnT0)
                stage_c(1, qkT1, v1, attnT1, needs_bulk=True)
                push_d(1, attnT1)
                drain_all()

    nc.compile()
    return nc


_NC_CACHE = {}


def _get_nc():
    if "nc" not in _NC_CACHE:
        _NC_CACHE["nc"] = build_nc()
    return _NC_CACHE["nc"]


def kernel(x, w_qkv, b_qkv, w_proj, b_proj):
    x = np.asarray(x, dtype=np.float32)
    w_qkv = np.asarray(w_qkv, dtype=np.float32)
    b_qkv = np.asarray(b_qkv, dtype=np.float32)
    w_proj = np.asarray(w_proj, dtype=np.float32)
    b_proj = np.asarray(b_proj, dtype=np.float32)

    nc = _get_nc()
    in_maps = [
        {
            "x": np.ascontiguousarray(x[i * BL:(i + 1) * BL]),
            "w_qkv": w_qkv,
            "b_qkv": b_qkv,
            "w_proj": w_proj,
            "b_proj": b_proj,
        }
        for i in range(NCORES)
    ]
    res = run_bass_kernel_spmd(nc, in_maps, list(range(NCORES)))
    return np.concatenate([res.results[i]["out"] for i in range(NCORES)], axis=0)
